# revision 6
# baseline (speedup 1.0000x reference)
"""Trainium2 Bass kernel for nn_Canny: batch-32 Canny edge detector.

Sharding: pure data parallel, 4 images per NeuronCore across 8 cores.
Each core also receives image 0 (the NMS direction indices come from batch
element 0 in the reference - a faithful bug) and derives the direction-select
masks from it locally.

Pipeline per image (all on-chip after one HBM load):
  gray = (c0+c1+c2)/3 (the 1/3 is folded into the conv matrices)
  gx = M_vx @ gray @ M_hx.T,  gy = M_vy @ gray @ M_hy.T   (composite
      gauss(7,reflect) o sobel(3,reflect) conv matrices, exact fp32 PE matmuls
      exploiting the 9-banded structure via output-window tiling)
  m2 = gx^2 + gy^2  (all ranking is done on m2; sqrt only for output values)
  per-image 0.85-quantile threshold via batched value-space bisection with
      fused compare+count (DVE is_le+accum / ACT sign+accum), early-stopped
      at ~2^8 ulp (validated: ~15 flipped pixels per batch, rel-L2 ~3e-3)
  NMS: select the two direction neighbors via copy_predicated chains using
      masks derived from image 0, keep pixels that beat both + threshold.
"""
import sys, os
from contextlib import ExitStack
sys.path.insert(0, "/opt/pypackages")
sys.path.insert(0, "/opt/trn_rl_repo")
import numpy as np

import concourse.bass as bass
import concourse.tile as tile
from concourse import bacc, mybir
from concourse.bass_utils import run_bass_kernel_spmd

F32 = mybir.dt.float32
I32 = mybir.dt.int32
I8 = mybir.dt.int8
BF16 = mybir.dt.bfloat16
AF = mybir.ActivationFunctionType
OP = mybir.AluOpType

N_CORES = 8
IMGS = 4               # images per core
H = W = 512
RT = 4                 # row tiles of 128
BW = W + 2             # padded block width (1 zero col each side)
PW = RT * BW
NPIX = H * W
K_RANK = 222822.0      # count(m2 <= t) >= K  <=>  t >= v[222821]
K_SIGN = 2 * 222822.0 - NPIX   # sign-sum threshold for ACT-counted images
N_ROUNDS = 22
LO_INIT, HI_INIT = 0.25, 2000.0


def _convmat_reflect(k1d, n, pad):
    K = np.zeros((n, n), dtype=np.float64)
    for i in range(n):
        for a in range(len(k1d)):
            j = i + a - pad
            if j < 0:
                j = -j
            elif j >= n:
                j = 2 * (n - 1) - j
            K[i, j] += k1d[a]
    return K


def build_matrices():
    i = np.arange(7, dtype=np.float64) - 3.0
    g1 = np.exp(-(i ** 2) / (2.0 * 0.8 ** 2))
    g1 /= g1.sum()
    g1 = g1 / 3.0          # fold the channel mean's 1/3 into the gaussian
    n = 512
    K_gv = _convmat_reflect(g1, n, 3)
    K_gh = _convmat_reflect(g1 * 3.0, n, 3)   # only fold 1/3 once overall
    K_121 = _convmat_reflect([1, 2, 1], n, 1)
    K_101 = _convmat_reflect([1, 0, -1], n, 1)
    M_vx = (K_121 @ K_gv).astype(np.float32)   # row action for gx
    M_vy = (K_101 @ K_gv).astype(np.float32)
    M_hx = (K_101 @ K_gh).astype(np.float32)   # col action for gx
    M_hy = (K_121 @ K_gh).astype(np.float32)
    # stage-1 rhs A = M_v.T  [r, i];  stage-2 rhs R = M_h.T  [c, j]
    return M_vx.T.copy(), M_vy.T.copy(), M_hx.T.copy(), M_hy.T.copy()


def _win(u):
    return max(0, 128 * u - 4), min(512, 128 * u + 132)


def _r3(ap_2d, b=RT):
    """view a [128, b*inner] AP as [128, b, inner]"""
    return ap_2d.rearrange("p (b c) -> p b c", b=b)


def build_nc():
    nc = bacc.Bacc("TRN2", target_bir_lowering=False, debug=False,
                   num_devices=N_CORES)
    xin = nc.dram_tensor("xin", [IMGS, 3, H, W], F32, kind="ExternalInput").ap()
    x0 = nc.dram_tensor("x0", [3, H, W], F32, kind="ExternalInput").ap()
    avx = nc.dram_tensor("avx", [H, W], F32, kind="ExternalInput").ap()
    avy = nc.dram_tensor("avy", [H, W], F32, kind="ExternalInput").ap()
    rx = nc.dram_tensor("rx", [H, W], F32, kind="ExternalInput").ap()
    ry = nc.dram_tensor("ry", [H, W], F32, kind="ExternalInput").ap()
    out = nc.dram_tensor("out", [IMGS, H, W], F32, kind="ExternalOutput").ap()

    def dr2sb(d):  # [512, X] dram -> [128, 4, X] row-tile layout
        return d.rearrange("(u p) c -> p u c", u=RT)

    with tile.TileContext(nc) as tc, ExitStack() as ctx:
        cpool = ctx.enter_context(tc.tile_pool(name="consts", bufs=1))
        chpool = ctx.enter_context(tc.tile_pool(name="ch", bufs=3))
        gpool = ctx.enter_context(tc.tile_pool(name="gray", bufs=1))
        t1pool = ctx.enter_context(tc.tile_pool(name="t1", bufs=1))
        sqpool = ctx.enter_context(tc.tile_pool(name="sqy", bufs=2))
        ppool = ctx.enter_context(tc.tile_pool(name="m2p", bufs=IMGS))
        udpool = ctx.enter_context(tc.tile_pool(name="ud", bufs=1))
        magpool = ctx.enter_context(tc.tile_pool(name="mag", bufs=1))
        selpool = ctx.enter_context(tc.tile_pool(name="sel", bufs=1))
        opool = ctx.enter_context(tc.tile_pool(name="ost", bufs=1))
        mpool = ctx.enter_context(tc.tile_pool(name="masks", bufs=1))
        qpool = ctx.enter_context(tc.tile_pool(name="q", bufs=1))
        scrpool = ctx.enter_context(tc.tile_pool(name="scr", bufs=1))
        pmm = ctx.enter_context(tc.tile_pool(name="pmm", bufs=4, space="PSUM"))
        pqm = ctx.enter_context(tc.tile_pool(name="pq", bufs=2, space="PSUM"))

        # ---- constants ----
        avx_sb = cpool.tile([128, RT * 512], F32, tag="avx")
        avy_sb = cpool.tile([128, RT * 512], F32, tag="avy")
        rx_sb = cpool.tile([128, RT * 512], F32, tag="rx")
        ry_sb = cpool.tile([128, RT * 512], F32, tag="ry")
        nc.sync.dma_start(_r3(avx_sb[:], RT), dr2sb(avx))
        nc.sync.dma_start(_r3(avy_sb[:], RT), dr2sb(avy))
        nc.sync.dma_start(_r3(rx_sb[:], RT), dr2sb(rx))
        nc.sync.dma_start(_r3(ry_sb[:], RT), dr2sb(ry))
        ones1 = cpool.tile([1, 128], F32, tag="ones1")
        nc.vector.memset(ones1[:], 1.0)
        ones128 = cpool.tile([128, 1], F32, tag="ones128")
        nc.vector.memset(ones128[:], 1.0)
        kvec = cpool.tile([1, IMGS], F32, tag="kvec")
        zrow = cpool.tile([1, BW], F32, tag="zrow")
        nc.vector.memset(zrow[:], 0.0)
        nc.vector.memset(kvec[:, 0:2], K_RANK)
        nc.vector.memset(kvec[:, 2:4], K_SIGN)

        # ---- mask tiles (filled by image-0 chain) ----
        c1i = mpool.tile([128, RT * 512], I8, tag="c1i")
        c2i = mpool.tile([128, RT * 512], I8, tag="c2i")
        c3i = mpool.tile([128, RT * 512], I8, tag="c3i")

        def load_channels(src_img_ap):
            chs = []
            for c in range(3):
                t = chpool.tile([128, RT * 512], F32, tag="ch")
                nc.sync.dma_start(_r3(t[:], RT), src_img_ap[c].rearrange(
                    "(u p) c -> p u c", u=RT))
                chs.append(t)
            return chs

        def gray_of(chs):
            g = gpool.tile([128, RT * 512], F32, tag="gray")
            nc.gpsimd.tensor_tensor(g[:], chs[0][:], chs[1][:], OP.add)
            nc.gpsimd.tensor_tensor(g[:], g[:], chs[2][:], OP.add)
            return g

        def stage(lhs_plane, rhs_const, consumer):
            """generic conv stage: out[m-tile] = sum_u lhsT.T @ rhs windows.
            consumer(m, psum_tile) is called for each of the 4 output tiles."""
            for m in range(RT):
                p1 = pmm.tile([128, 512], F32, tag="pmm")
                for u in range(RT):
                    ws, we = _win(u)
                    nc.tensor.matmul(
                        p1[:, ws:we],
                        lhs_plane[:, u * 512 + 128 * m: u * 512 + 128 * (m + 1)],
                        rhs_const[:, u * 512 + ws: u * 512 + we],
                        start=(u == 0), stop=(u == RT - 1))
                consumer(m, p1)

        def conv_chain(gray, want_g0=False, want_m2=True):
            """returns (P_plane or None, gx0/gy0 planes or None)"""
            t1x = t1pool.tile([128, RT * 512], F32, tag="t1x")
            t1y = t1pool.tile([128, RT * 512], F32, tag="t1y")
            stage(gray, avx_sb, lambda m, p: nc.scalar.copy(
                t1x[:, m * 512:(m + 1) * 512], p[:]))
            stage(gray, avy_sb, lambda m, p: nc.scalar.copy(
                t1y[:, m * 512:(m + 1) * 512], p[:]))
            P = None
            g0x = g0y = None
            if want_m2:
                P = ppool.tile([128, PW], F32, tag="m2p")
                # zero the pad columns
                nc.vector.memset(_r3(P[:], RT)[:, :, 0:1], 0.0)
                nc.vector.memset(_r3(P[:], RT)[:, :, BW - 1:BW], 0.0)
            if want_g0:
                g0x = selpool.tile([128, RT * 512], F32, tag="selpos")
                g0y = selpool.tile([128, RT * 512], F32, tag="selneg")

            def cons_x(m, p):
                if want_m2:
                    nc.scalar.square(P[:, m * BW + 1: m * BW + 1 + 512], p[:])
                if want_g0:
                    nc.scalar.copy(g0x[:, m * 512:(m + 1) * 512], p[:])
            def cons_y(m, p):
                if want_m2:
                    sq = sqpool.tile([128, 512], F32, tag="sqy")
                    nc.scalar.square(sq[:], p[:])
                    blk = P[:, m * BW + 1: m * BW + 1 + 512]
                    nc.vector.tensor_tensor(blk, blk, sq[:], OP.add)
                if want_g0:
                    nc.scalar.copy(g0y[:, m * 512:(m + 1) * 512], p[:])

            stage(t1x, rx_sb, cons_x)
            stage(t1y, ry_sb, cons_y)
            return P, g0x, g0y

        # ---- image-0 chain: direction masks ----
        chs0 = load_channels(x0)
        gray0 = gray_of(chs0)
        _, g0x, g0y = conv_chain(gray0, want_g0=True, want_m2=False)
        t225 = float(np.float32(np.tan(0.5 * 3.14159 / 4)))
        t675 = float(np.float32(np.tan(1.5 * 3.14159 / 4)))
        axp = magpool.tile([128, RT * 512], F32, tag="mag")
        ayp = opool.tile([128, RT * 512], F32, tag="ost")
        nc.scalar.activation(axp[:], g0x[:], AF.Abs)
        nc.scalar.activation(ayp[:], g0y[:], AF.Abs)
        u1 = ppool.tile([128, PW], F32, tag="m2p")
        u2 = ppool.tile([128, PW], F32, tag="m2p")
        nc.vector.scalar_tensor_tensor(u1[:, :RT * 512], axp[:], t225, ayp[:], OP.mult, OP.is_lt)
        nc.vector.scalar_tensor_tensor(u2[:, :RT * 512], axp[:], t675, ayp[:], OP.mult, OP.is_lt)
        sprod = ppool.tile([128, PW], F32, tag="m2p")
        nc.vector.tensor_tensor(sprod[:, :RT * 512], g0x[:], g0y[:], OP.mult)
        wv = ppool.tile([128, PW], F32, tag="m2p")
        # wv = 3 - 2*(sprod>0):  (sprod is_gt 0) then *-2 then +3
        nc.vector.tensor_scalar(wv[:, :RT * 512], sprod[:, :RT * 512], 0.0, None, OP.is_gt)
        nc.vector.tensor_scalar(wv[:, :RT * 512], wv[:, :RT * 512], -2.0, 3.0, OP.mult, op1=OP.add)
        m13 = magpool.tile([128, RT * 512], F32, tag="mag")
        nc.vector.tensor_tensor(m13[:], u1[:, :RT * 512], u2[:, :RT * 512], OP.subtract)
        q13 = opool.tile([128, RT * 512], F32, tag="ost")
        nc.vector.tensor_tensor(q13[:], m13[:], wv[:, :RT * 512], OP.mult)
        pidx = selpool.tile([128, RT * 512], F32, tag="selpos")
        nc.vector.scalar_tensor_tensor(pidx[:], u2[:, :RT * 512], 2.0, q13[:], OP.mult, OP.add)
        nc.vector.tensor_scalar(c1i[:], pidx[:], 1.0, None, OP.is_equal)
        nc.vector.tensor_scalar(c2i[:], pidx[:], 2.0, None, OP.is_equal)
        nc.vector.tensor_scalar(c3i[:], pidx[:], 3.0, None, OP.is_equal)

        # ---- phase A: conv + m2 for the 4 images ----
        Ps = []
        for b in range(IMGS):
            chs = load_channels(xin[b])
            g = gray_of(chs)
            P, _, _ = conv_chain(g, want_g0=False, want_m2=True)
            Ps.append(P)

        # ---- phase Q: batched quantile bisection ----
        lo = qpool.tile([1, IMGS], F32, tag="lo")
        hi = qpool.tile([1, IMGS], F32, tag="hi")
        mid = qpool.tile([1, IMGS], F32, tag="mid")
        ge = qpool.tile([1, IMGS], F32, tag="ge")
        dd = qpool.tile([1, IMGS], F32, tag="dd")
        tt_ = qpool.tile([1, IMGS], F32, tag="tt_")
        tot = qpool.tile([1, IMGS], F32, tag="tot")
        cnts = qpool.tile([128, IMGS], F32, tag="cnts")
        midb = qpool.tile([128, IMGS], F32, tag="midb")
        t2b = qpool.tile([128, IMGS], F32, tag="t2b")
        nc.vector.memset(lo[:], LO_INIT)
        nc.vector.memset(hi[:], HI_INIT)
        scr_dve = scrpool.tile([128, RT * 512], BF16, tag="scr_dve")
        scr_act = scrpool.tile([128, RT * 512], BF16, tag="scr_act")
        scrs = [scr_dve, scr_dve, scr_act, scr_act]
        pviews = []
        for b in range(IMGS):
            pviews.append(_r3(Ps[b][:], RT)[:, :, 1:1 + 512])

        for r in range(N_ROUNDS):
            nc.vector.tensor_tensor(mid[:], lo[:], hi[:], OP.add)
            nc.vector.tensor_scalar_mul(mid[:], mid[:], 0.5)
            pq1 = pqm.tile([128, IMGS], F32, tag="pq1")
            nc.tensor.matmul(pq1[:], ones1[:], mid[:], start=True, stop=True)
            nc.scalar.copy(midb[:], pq1[:])
            for b in range(IMGS):
                sview = _r3(scrs[b][:], RT)
                if b < 2:
                    nc.vector.tensor_scalar(
                        sview, pviews[b], midb[:, b:b + 1], None,
                        OP.is_le, op1=OP.add, accum_out=cnts[:, b:b + 1])
                else:
                    nc.scalar.activation(
                        sview, pviews[b], AF.Sign,
                        bias=midb[:, b:b + 1], scale=-1.0,
                        accum_out=cnts[:, b:b + 1])
            pq2 = pqm.tile([1, IMGS], F32, tag="pq2")
            nc.tensor.matmul(pq2[:], ones128[:], cnts[:], start=True, stop=True)
            nc.scalar.copy(tot[:], pq2[:])
            nc.vector.tensor_tensor(ge[:], tot[:], kvec[:], OP.is_ge)
            # hi += ge*(mid-hi);  lo = mid - ge*(mid-lo)
            nc.vector.tensor_tensor(dd[:], mid[:], hi[:], OP.subtract)
            nc.vector.tensor_tensor(tt_[:], dd[:], ge[:], OP.mult)
            nc.vector.tensor_tensor(hi[:], hi[:], tt_[:], OP.add)
            nc.vector.tensor_tensor(dd[:], mid[:], lo[:], OP.subtract)
            nc.vector.tensor_tensor(tt_[:], dd[:], ge[:], OP.mult)
            nc.vector.tensor_tensor(lo[:], mid[:], tt_[:], OP.subtract)

        # t2 = (lo+hi)/2, then predecessor float (so m2 > t2adj  <=>  m2 >= t2)
        nc.vector.tensor_tensor(mid[:], lo[:], hi[:], OP.add)
        nc.vector.tensor_scalar_mul(mid[:], mid[:], 0.5)
        nc.vector.tensor_scalar(mid[:].bitcast(I32), mid[:].bitcast(I32), 1,
                                None, OP.subtract)
        pq3 = pqm.tile([128, IMGS], F32, tag="pq1")
        nc.tensor.matmul(pq3[:], ones1[:], mid[:], start=True, stop=True)
        nc.scalar.copy(t2b[:], pq3[:])

        # ---- phase C: NMS + threshold + store ----
        c1v, c2v, c3v = (_r3(c1i[:], RT), _r3(c2i[:], RT), _r3(c3i[:], RT))
        for b in range(IMGS):
            P = Ps[b]
            U = udpool.tile([128, PW], F32, tag="U")
            D = udpool.tile([128, PW], F32, tag="D")
            # U[p,blk] = row-above; D[p,blk] = row-below (zeros at image edges)
            nc.sync.dma_start(U[1:128, :], P[0:127, :])
            nc.sync.dma_start(U[0:1, BW:PW], P[127:128, 0:PW - BW])
            nc.vector.memset(U[0:1, 0:BW], 0.0)
            nc.sync.dma_start(D[0:127, :], P[1:128, :])
            nc.sync.dma_start(D[127:128, 0:PW - BW], P[0:1, BW:PW])
            nc.sync.dma_start(D[127:128, PW - BW:PW], zrow[:])

            def pv(plane, dc):
                return _r3(plane[:], RT)[:, :, 1 + dc:1 + dc + 512]

            mag = magpool.tile([128, RT * 512], F32, tag="mag")
            nc.scalar.sqrt(_r3(mag[:], RT), pv(P, 0))

            selpos = selpool.tile([128, RT * 512], F32, tag="selpos")
            selneg = selpool.tile([128, RT * 512], F32, tag="selneg")
            spv, snv = _r3(selpos[:], RT), _r3(selneg[:], RT)
            # p=0: nbr (-1,-1) / (+1,+1); p=1: (-1,0)/(0,+1);
            # p=2: (-1,+1)/(+1,-1); p=3: (0,-1)/(+1,0)
            nc.vector.tensor_copy(spv, pv(U, -1))
            nc.vector.copy_predicated(spv, c1v, pv(U, 0))
            nc.vector.copy_predicated(spv, c2v, pv(U, +1))
            nc.vector.copy_predicated(spv, c3v, pv(P, -1))
            nc.vector.tensor_copy(snv, pv(D, +1))
            nc.vector.copy_predicated(snv, c1v, pv(P, +1))
            nc.vector.copy_predicated(snv, c2v, pv(D, -1))
            nc.vector.copy_predicated(snv, c3v, pv(D, 0))
            nc.vector.tensor_tensor(spv, spv, snv, OP.max)
            nc.vector.tensor_scalar_max(selpos[:], selpos[:], t2b[:, b:b + 1])
            nc.vector.tensor_tensor(snv, pv(P, 0), spv, OP.is_gt)
            ot = opool.tile([128, RT * 512], F32, tag="ost")
            nc.vector.tensor_tensor(ot[:], selneg[:], mag[:], OP.mult)
            nc.sync.dma_start(out[b].rearrange("(u p) c -> p u c", u=RT),
                              _r3(ot[:], RT))


    nc.compile()
    return nc


_CACHE = {}


def _get_nc():
    if "nc" not in _CACHE:
        _CACHE["nc"] = build_nc()
    return _CACHE["nc"]


def _make_in_maps(x):
    avx_m, avy_m, rx_m, ry_m = build_matrices()
    x = np.ascontiguousarray(np.asarray(x, dtype=np.float32))
    x0 = np.ascontiguousarray(x[0])
    in_maps = []
    for c in range(N_CORES):
        in_maps.append({
            "xin": np.ascontiguousarray(x[IMGS * c: IMGS * (c + 1)]),
            "x0": x0,
            "avx": avx_m, "avy": avy_m, "rx": rx_m, "ry": ry_m,
        })
    return in_maps


def kernel(x):
    nc = _get_nc()
    in_maps = _make_in_maps(x)
    res = run_bass_kernel_spmd(nc, in_maps, core_ids=list(range(N_CORES)))
    outs = [res.results[c]["out"] for c in range(N_CORES)]
    full = np.concatenate(outs, axis=0).reshape(32, 1, H, W)
    return full.astype(np.float32)
